# revision 20
# baseline (speedup 1.0000x reference)
"""MoE top-1 routing kernel for Trainium2 (8 NeuronCores).

Math (matches the reference):
    logits = x @ gate_w + gate_b            # [N, E]
    assign = argmax(logits, -1)             # top-1 expert per token
    out[t] = relu(x[t] @ w1[e] + b1[e]) @ w2[e] + b2[e]   where e = assign[t]

The gate is a tiny (4096x1024x8) matmul computed on the host in float64 (the
smallest top1-top2 logit gap in this regime is ~2e-4, orders of magnitude
above fp32 rounding, so the argmax is unambiguous). Tokens are grouped by
expert and dispatched to the cores holding that expert's weights; outputs are
scattered back to token order on the host.

Device sharding (TP4 expert-quads): experts are split into two quads of 4
(snake-ordered by token count so slotwise maxima are minimal); each quad maps
to 4 cores, and each core holds a DFF/4 slice of all 4 experts in its quad
(16MB bf16 weights per core). Every core processes all its quad's tokens
through its DFF-quarter; relu is elementwise so layer-1 slices are
independent, and layer 2 produces partial sums over the quarter which the
host adds (4 partials per token). Slots are ordered by ASCENDING capacity so
the kernel's DMA-gating prefix (first w1 block + first token chunk) is
minimal.

Per-core device kernel (bf16 matmul datapath, fp32 PSUM accumulation):
    layer1: hT[m*128+p, c] = relu(sum_k w1h[k,: x m,:]^T @ xT[k,: x c] + b1h)
    layer2: yT[m2*128+p, c] = sum_k2 w2h[k2,: x m2,:]^T @ hT[k2,: x c]
Contraction stays on SBUF partitions, tokens on the free dim: no on-device
transposes. The host pre-tiles weights so every DMA is contiguous.

Schedule notes (from NTFF traces of this exact kernel):
  * The PE stream itself runs at ~99% occupancy at the bf16 roofline
    (1 row/cycle @2.4GHz); all the recoverable time is at the edges.
  * ALL inputs ride the single Sync HWDGE queue in consumption order. A
    HWDGE queue stripes one DMA over all 16 SDMA engines and drains FIFO,
    so later DMAs on the same queue never delay earlier ones — but a
    SECOND active queue steals ~half the engines via per-packet
    round-robin. The previous revision loaded non-gating chunks on the
    scalar queue in parallel and pushed the gating chunk0 out to ~13.3us;
    with strict single-queue ordering the prefix (w1q[0] 256KB + 288-token
    chunk0 0.59MB) lands at ~10.5us.
  * Warmup dummy matmuls bridge the framework preamble (~7.7us) to first
    data arrival AND pre-warm the PE HAM clock gate (1.2->2.4GHz after
    ~3.4us of sustained PE busy). Count is tuned so warmup ends right as
    chunk0 lands.
  * The kernel-final store tail = last ACT + DMA issue (~0.6us) + DMA
    completion latency (~1.4us) + NEFF teardown (~3us). The final PSUM
    group is a 96-token sliver so the post-last-matmul ACT is tiny, and
    the last two stores ride the two HWDGE queues in parallel.
"""

import numpy as np
import ml_dtypes

BF16 = np.dtype(ml_dtypes.bfloat16)

N_TOK, D, DFF, E = 4096, 1024, 4096, 8
P = 128
KD = D // P  # 8 contraction chunks of the d dimension
MQ = (DFF // 4) // P  # 8 dff-quarter blocks (TP4 layer1 out / layer2 contraction)

# test.py hooks: set TRACE=True (after installing the NTFF hook) to profile.
TRACE = False
TRACE_CORES = None
LAST_RESULT = None

_PROGRAM_CACHE = {}

WARMUP_MMS = 17  # 256-col cold matmuls @~213ns each; >=3.4us total busy so
# the HAM clock gate is fully warm before the real stream starts (a shorter
# warmup measured: real stream started cold and lost ~1.7us at half rate)
FINAL_SLIVER = 64  # token width of the kernel-final PSUM group


def _pad_cap(n):
    """Token capacity: multiple of 8, >=256."""
    return max(256, -(-n // 8) * 8)


def _chunk_sizes(C):
    """Split C tokens into moving-dim chunks <=512 (PSUM bank), balanced."""
    n = -(-C // 512)
    base, rem = divmod(C, n)
    return [base + (1 if i < rem else 0) for i in range(n)]


def _chunk_sizes_slot0(C):
    """Slot-0 chunks: two small 144-token gating chunks first (loaded on the
    two HWDGE queues in parallel — one queue tops out at ~215GB/s), rest
    balanced."""
    if C <= 288:
        h = (C // 2 + 7) // 8 * 8
        return [h, C - h] if C > h else [C]
    return [144, 144] + _chunk_sizes(C - 288)


def _build_program_tp4(caps):
    """TP4: 4 cores per expert-quad; core holds a DFF/4 slice of 4 experts."""
    import concourse.mybir as mybir
    import concourse.tile as tile
    from concourse import bacc

    f32 = mybir.dt.float32
    bf16 = mybir.dt.bfloat16
    AF = mybir.ActivationFunctionType

    S = list(caps)
    chunks = [_chunk_sizes_slot0(S[0])] + [_chunk_sizes(c) for c in S[1:]]
    # L2 re-chunks coarsely (<=512): the fine slot-0 gating chunks only
    # matter for layer 1's DMA overlap; extra chunks cost NX dispatch
    chunks2 = [_chunk_sizes(c) for c in S]

    nc = bacc.Bacc("TRN2", target_bir_lowering=False, debug=False, num_devices=E)

    # slot 0 arrives chunk-major (one dram tensor per token chunk) so the
    # real stream is gated on w1q[0] + chunk0 only.
    xt0c_d = [
        nc.dram_tensor(f"xt0c{ci}", [P, KD * tn], bf16, kind="ExternalInput").ap()
        for ci, tn in enumerate(chunks[0])
    ]
    xt_d = [None] + [
        nc.dram_tensor(f"xt{s}", [P, KD * S[s]], bf16, kind="ExternalInput").ap()
        for s in range(1, 4)
    ]
    w1q_d = nc.dram_tensor("w1q", [4 * MQ, P, D], bf16, kind="ExternalInput").ap()
    b1q_d = nc.dram_tensor("b1q", [P, 4 * MQ], f32, kind="ExternalInput").ap()
    w2q_d = nc.dram_tensor("w2q", [4 * KD, P, MQ * P], bf16, kind="ExternalInput").ap()
    yt_d = [
        nc.dram_tensor(f"yt{s}", [KD, P, S[s]], bf16, kind="ExternalOutput").ap()
        for s in range(4)
    ]

    with tile.TileContext(nc) as tc:
        with (
            tc.tile_pool(name="xt_pool", bufs=1) as xt_pool,
            tc.tile_pool(name="ht_pool", bufs=1) as ht_pool,
            # deep weight prefetch: shallow pools gate DMA issue on
            # buffer-free semaphores and starve LDWEIGHTS
            tc.tile_pool(name="w1_pool", bufs=20) as w1_pool,
            tc.tile_pool(name="w2_pool", bufs=16) as w2_pool,
            # 8 y bufs: with 4, Scalar blocked on gpsimd store-completion
            # semaphores and the stalled ACTs backed up into TensorE
            tc.tile_pool(name="y_pool", bufs=8) as y_pool,
            tc.tile_pool(name="bias_pool", bufs=1) as bias_pool,
            tc.tile_pool(name="psum", bufs=8, space="PSUM") as psum_pool,
        ):
            # PE p-state warm-up AND idle-bridge (see module docstring)
            warm_sb = bias_pool.tile([P, 256], bf16)
            nc.vector.memset(warm_sb[:], 0.0)
            warm_ps = psum_pool.tile([P, 256], f32, tag="ps")
            for i in range(WARMUP_MMS):
                nc.tensor.matmul(
                    warm_ps[:],
                    lhsT=warm_sb[:, :P],
                    rhs=warm_sb[:],
                    start=(i == 0),
                    stop=(i == WARMUP_MMS - 1),
                )

            xt0c_sb = [
                xt_pool.tile([P, KD * tn], bf16, name=f"xt0c_sb{ci}")
                for ci, tn in enumerate(chunks[0])
            ]
            xt_sb = [None] + [
                xt_pool.tile([P, KD * S[s]], bf16, name=f"xt_sb{s}")
                for s in range(1, 4)
            ]
            ht_sb = [
                ht_pool.tile([P, MQ * S[s]], bf16, name=f"ht_sb{s}") for s in range(4)
            ]

            # The gating prefix rides BOTH HWDGE queues (each ~215GB/s),
            # byte-balanced: sync: w1q[0] + chunk0a (then the weight
            # stream); scalar: chunk0b + b1q + remaining slot-0 chunks.
            w1_first = w1_pool.tile([P, D], bf16, tag="w1")
            nc.sync.dma_start(w1_first[:], w1q_d[0])
            if len(chunks[0]) > 1:
                nc.scalar.dma_start(xt0c_sb[1][:], xt0c_d[1][:])
            nc.sync.dma_start(xt0c_sb[0][:], xt0c_d[0][:])
            b1q_sb = bias_pool.tile([P, 4 * MQ], f32)
            nc.scalar.dma_start(b1q_sb[:], b1q_d[:])
            for ci in range(2, len(chunks[0])):
                nc.scalar.dma_start(xt0c_sb[ci][:], xt0c_d[ci][:])

            def l1_chunk(e, m, w1_sb, s, rhs_fn, tn, t0):
                C = S[s]
                ps = psum_pool.tile([P, 512], f32, tag="ps")
                for k in range(KD):
                    nc.tensor.matmul(
                        ps[:, :tn],
                        lhsT=w1_sb[:, k * P : (k + 1) * P],
                        rhs=rhs_fn(k),
                        start=(k == 0),
                        stop=(k == KD - 1),
                    )
                nc.scalar.activation(
                    ht_sb[s][:, m * C + t0 : m * C + t0 + tn],
                    ps[:, :tn],
                    AF.Relu,
                    bias=b1q_sb[:, e * MQ + m : e * MQ + m + 1],
                )

            def layer2(e, m2, w2_sb, s, last=False, sync_store=False):
                C, t0 = S[s], 0
                for tn in chunks2[s]:
                    if last and t0 + tn == C:
                        # final block: a small sliver group last, so the
                        # post-last-matmul ACT+store chain is minimal; the
                        # two stores ride the two HWDGE queues in parallel
                        h1 = tn - FINAL_SLIVER if tn > FINAL_SLIVER else tn // 2
                        for j, (lo, hi) in enumerate(
                            ((t0, t0 + h1), (t0 + h1, t0 + tn))
                        ):
                            w = hi - lo
                            psh = psum_pool.tile([P, 512], f32, tag="ps")
                            for k2 in range(MQ):
                                nc.tensor.matmul(
                                    psh[:, :w],
                                    lhsT=w2_sb[:, k2 * P : (k2 + 1) * P],
                                    rhs=ht_sb[s][:, k2 * C + lo : k2 * C + hi],
                                    start=(k2 == 0),
                                    stop=(k2 == MQ - 1),
                                )
                            yts = y_pool.tile([P, 512], bf16, tag="yt")
                            nc.scalar.activation(yts[:, :w], psh[:, :w], AF.Identity)
                            # the very last store rides SCALAR: Scalar-seq
                            # waits out its ~3.4us completion latency while
                            # Sync-seq runs the queue-drain waits and
                            # semaphore teardown in parallel (measured: the
                            # other order serializes them, +1.5us)
                            q = nc.sync if j == 0 else nc.scalar
                            q.dma_start(yt_d[s][m2][:, lo:hi], yts[:, :w])
                    else:
                        ps2 = psum_pool.tile([P, 512], f32, tag="ps")
                        for k2 in range(MQ):
                            nc.tensor.matmul(
                                ps2[:, :tn],
                                lhsT=w2_sb[:, k2 * P : (k2 + 1) * P],
                                rhs=ht_sb[s][:, k2 * C + t0 : k2 * C + t0 + tn],
                                start=(k2 == 0),
                                stop=(k2 == MQ - 1),
                            )
                        yt_sb = y_pool.tile([P, 512], bf16, tag="yt")
                        nc.scalar.activation(yt_sb[:, :tn], ps2[:, :tn], AF.Identity)
                        q = nc.sync if sync_store else nc.gpsimd
                        q.dma_start(yt_d[s][m2][:, t0 : t0 + tn], yt_sb[:, :tn])
                    t0 += tn

            def load_xt_piece(s, pc):
                # bulk token loads ride SCALAR so the sync queue carries the
                # weight stream UNINTERRUPTED (xt injected into the sync
                # stream starves w1 — slot-0's phase has less w1 slack than
                # one xt load; measured 0.3-2.4us PE gaps). The scalar
                # issues are THROTTLED by their position in the Scalar-seq
                # instruction stream: placed after an L1 ACT, the in-order
                # sequencer only reaches the DMA once that ACT's PSUM group
                # completed — so early pieces can't flood the prefix window
                # (measured 1.5us gap when unthrottled).
                kq = 4 * S[s]
                nc.scalar.dma_start(
                    xt_sb[s][:, pc * kq : (pc + 1) * kq],
                    xt_d[s][:, pc * kq : (pc + 1) * kq],
                )

            # xt1 is needed soonest: it queues on scalar right behind the
            # slot-0 chunks, ungated. xt2/xt3 gate at L1-ACT positions.
            if xt_sb[1] is not None:
                load_xt_piece(1, 0)
                load_xt_piece(1, 1)
            xt_insert = {
                (0, 4): (2, 0), (0, 6): (2, 1),
                (1, 2): (3, 0), (1, 4): (3, 1),
            }
            for e in range(4):
                for m in range(MQ):
                    if e == 0 and m == 0:
                        w1_sb = w1_first
                    else:
                        w1_sb = w1_pool.tile([P, D], bf16, tag="w1")
                        nc.sync.dma_start(w1_sb[:], w1q_d[e * MQ + m])
                    ins = xt_insert.get((e, m))
                    if ins is not None:
                        load_xt_piece(*ins)
                    C, te = S[e], 0
                    for ci, tn in enumerate(chunks[e]):
                        if e == 0:
                            xc = xt0c_sb[ci]
                            rhs_fn = lambda k, xc=xc, tn=tn: xc[
                                :, k * tn : (k + 1) * tn
                            ]
                        else:
                            xe = xt_sb[e]
                            rhs_fn = lambda k, xe=xe, C=C, te=te, tn=tn: xe[
                                :, k * C + te : k * C + te + tn
                            ]
                        l1_chunk(e, m, w1_sb, e, rhs_fn, tn, te)
                        te += tn

            for e in range(3):
                for m2 in range(KD):
                    w2_sb = w2_pool.tile([P, MQ * P], bf16, tag="w2")
                    nc.sync.dma_start(w2_sb[:], w2q_d[e * KD + m2])
                    layer2(e, m2, w2_sb, e)
            # last expert: prefetch all 8 w2 blocks, then run its stores on
            # the (now idle) Sync HWDGE queue — the gpsimd SWDGE path has a
            # ~5us issue->drain latency that otherwise sits on the tail
            w2_last = []
            for m2 in range(KD):
                w2_sb = w2_pool.tile([P, MQ * P], bf16, tag="w2")
                nc.sync.dma_start(w2_sb[:], w2q_d[3 * KD + m2])
                w2_last.append(w2_sb)
            for m2 in range(KD):
                layer2(3, m2, w2_last[m2], 3, last=(m2 == KD - 1), sync_store=True)

    nc.compile()
    return nc


MQ8 = (DFF // 8) // P  # 4 dff-eighth blocks (TP8 layer1 out / layer2 contraction)


def _build_program_tp8(caps):
    """TP8: every core holds a DFF/8 slice of ALL 8 experts and processes
    ALL tokens through it. No slot-capacity padding at all (per-expert pad
    to 8 only): per-core rows = 64 * sum(caps) ~ 263.7k vs TP4's 267.3k.
    Host sums 8 partials per token. Layer-2 PSUM groups are only 4 deep
    (contraction DFF/8 = 512), so the PSUM->SBUF copies run on the
    otherwise-idle Vector engine and stores alternate scalar/gpsimd to keep
    every sequencer under the ~0.9us/group PE pace."""
    import concourse.mybir as mybir
    import concourse.tile as tile
    from concourse import bacc

    f32 = mybir.dt.float32
    bf16 = mybir.dt.bfloat16
    AF = mybir.ActivationFunctionType

    S = list(caps)
    NE = len(S)
    chunks = [_chunk_sizes_slot0(S[0])] + [_chunk_sizes(c) for c in S[1:]]
    chunks2 = [_chunk_sizes(c) for c in S]

    nc = bacc.Bacc("TRN2", target_bir_lowering=False, debug=False, num_devices=E)

    xt0c_d = [
        nc.dram_tensor(f"xt0c{ci}", [P, KD * tn], bf16, kind="ExternalInput").ap()
        for ci, tn in enumerate(chunks[0])
    ]
    xt_d = [None] + [
        nc.dram_tensor(f"xt{s}", [P, KD * S[s]], bf16, kind="ExternalInput").ap()
        for s in range(1, NE)
    ]
    w1q_d = nc.dram_tensor("w1q", [NE * MQ8, P, D], bf16, kind="ExternalInput").ap()
    b1q_d = nc.dram_tensor("b1q", [P, NE * MQ8], f32, kind="ExternalInput").ap()
    w2q_d = nc.dram_tensor(
        "w2q", [NE * KD, P, MQ8 * P], bf16, kind="ExternalInput"
    ).ap()
    yt_d = [
        nc.dram_tensor(f"yt{s}", [KD, P, S[s]], bf16, kind="ExternalOutput").ap()
        for s in range(NE)
    ]

    with tile.TileContext(nc) as tc:
        with (
            tc.tile_pool(name="xt_pool", bufs=1) as xt_pool,
            tc.tile_pool(name="ht_pool", bufs=1) as ht_pool,
            tc.tile_pool(name="w1_pool", bufs=20) as w1_pool,
            tc.tile_pool(name="w2_pool", bufs=16) as w2_pool,
            tc.tile_pool(name="y_pool", bufs=8) as y_pool,
            tc.tile_pool(name="bias_pool", bufs=1) as bias_pool,
            tc.tile_pool(name="psum", bufs=8, space="PSUM") as psum_pool,
        ):
            warm_sb = bias_pool.tile([P, 256], bf16)
            nc.vector.memset(warm_sb[:], 0.0)
            warm_ps = psum_pool.tile([P, 256], f32, tag="ps")
            for i in range(WARMUP_MMS):
                nc.tensor.matmul(
                    warm_ps[:],
                    lhsT=warm_sb[:, :P],
                    rhs=warm_sb[:],
                    start=(i == 0),
                    stop=(i == WARMUP_MMS - 1),
                )

            xt0c_sb = [
                xt_pool.tile([P, KD * tn], bf16, name=f"xt0c_sb{ci}")
                for ci, tn in enumerate(chunks[0])
            ]
            xt_sb = [None] + [
                xt_pool.tile([P, KD * S[s]], bf16, name=f"xt_sb{s}")
                for s in range(1, NE)
            ]
            ht_sb = [
                ht_pool.tile([P, MQ8 * S[s]], bf16, name=f"ht_sb{s}")
                for s in range(NE)
            ]

            # gating prefix split across both HWDGE queues
            w1_first = w1_pool.tile([P, D], bf16, tag="w1")
            nc.sync.dma_start(w1_first[:], w1q_d[0])
            if len(chunks[0]) > 1:
                nc.scalar.dma_start(xt0c_sb[1][:], xt0c_d[1][:])
            nc.sync.dma_start(xt0c_sb[0][:], xt0c_d[0][:])
            b1q_sb = bias_pool.tile([P, NE * MQ8], f32)
            nc.sync.dma_start(b1q_sb[:], b1q_d[:])
            for ci in range(2, len(chunks[0])):
                nc.scalar.dma_start(xt0c_sb[ci][:], xt0c_d[ci][:])
            # xt1 unthrottled on scalar right behind the e0 chunks; later
            # experts' loads are placed between ACT issues in the scalar
            # stream (the in-order Scalar-seq only reaches them once the
            # preceding expert's PSUM groups complete — a natural throttle
            # that keeps the early engine round-robin from starving w1)
            if NE > 1:
                nc.scalar.dma_start(
                    xt_sb[1][:, : 4 * S[1]], xt_d[1][:, : 4 * S[1]]
                )
                nc.scalar.dma_start(
                    xt_sb[1][:, 4 * S[1] :], xt_d[1][:, 4 * S[1] :]
                )

            def l1_chunk(e, m, w1_sb, rhs_fn, tn, t0):
                C = S[e]
                ps = psum_pool.tile([P, 512], f32, tag="ps")
                for k in range(KD):
                    nc.tensor.matmul(
                        ps[:, :tn],
                        lhsT=w1_sb[:, k * P : (k + 1) * P],
                        rhs=rhs_fn(k),
                        start=(k == 0),
                        stop=(k == KD - 1),
                    )
                nc.scalar.activation(
                    ht_sb[e][:, m * C + t0 : m * C + t0 + tn],
                    ps[:, :tn],
                    AF.Relu,
                    bias=b1q_sb[:, e * MQ8 + m : e * MQ8 + m + 1],
                )

            def layer2(e, m2, w2_sb, gidx, last=False, sync_store=False):
                C, t0 = S[e], 0
                for tn in chunks2[e]:
                    if last and t0 + tn == C:
                        h1 = tn - FINAL_SLIVER if tn > FINAL_SLIVER else tn // 2
                        for j, (lo, hi) in enumerate(
                            ((t0, t0 + h1), (t0 + h1, t0 + tn))
                        ):
                            w = hi - lo
                            psh = psum_pool.tile([P, 512], f32, tag="ps")
                            for k2 in range(MQ8):
                                nc.tensor.matmul(
                                    psh[:, :w],
                                    lhsT=w2_sb[:, k2 * P : (k2 + 1) * P],
                                    rhs=ht_sb[e][:, k2 * C + lo : k2 * C + hi],
                                    start=(k2 == 0),
                                    stop=(k2 == MQ8 - 1),
                                )
                            yts = y_pool.tile([P, 512], bf16, tag="yt")
                            nc.vector.tensor_copy(yts[:, :w], psh[:, :w])
                            q = nc.sync if j == 0 else nc.scalar
                            q.dma_start(yt_d[e][m2][:, lo:hi], yts[:, :w])
                    else:
                        ps2 = psum_pool.tile([P, 512], f32, tag="ps")
                        for k2 in range(MQ8):
                            nc.tensor.matmul(
                                ps2[:, :tn],
                                lhsT=w2_sb[:, k2 * P : (k2 + 1) * P],
                                rhs=ht_sb[e][:, k2 * C + t0 : k2 * C + t0 + tn],
                                start=(k2 == 0),
                                stop=(k2 == MQ8 - 1),
                            )
                        yt_sb = y_pool.tile([P, 512], bf16, tag="yt")
                        nc.vector.tensor_copy(yt_sb[:, :tn], ps2[:, :tn])
                        if sync_store:
                            q = nc.sync
                        else:
                            q = nc.scalar if gidx % 2 == 0 else nc.gpsimd
                        q.dma_start(yt_d[e][m2][:, t0 : t0 + tn], yt_sb[:, :tn])
                    t0 += tn

            # xt_{e+2} issue gated at (e, m=2) in the scalar stream
            xt_insert = {(e, 2): e + 2 for e in range(NE - 2)}
            for e in range(NE):
                for m in range(MQ8):
                    if e == 0 and m == 0:
                        w1_sb = w1_first
                    else:
                        w1_sb = w1_pool.tile([P, D], bf16, tag="w1")
                        nc.sync.dma_start(w1_sb[:], w1q_d[e * MQ8 + m])
                    s = xt_insert.get((e, m))
                    if s is not None:
                        nc.scalar.dma_start(
                            xt_sb[s][:, : 4 * S[s]], xt_d[s][:, : 4 * S[s]]
                        )
                        nc.scalar.dma_start(
                            xt_sb[s][:, 4 * S[s] :], xt_d[s][:, 4 * S[s] :]
                        )
                    C, te = S[e], 0
                    for ci, tn in enumerate(chunks[e]):
                        if e == 0:
                            xc = xt0c_sb[ci]
                            rhs_fn = lambda k, xc=xc, tn=tn: xc[
                                :, k * tn : (k + 1) * tn
                            ]
                        else:
                            xe = xt_sb[e]
                            rhs_fn = lambda k, xe=xe, C=C, te=te, tn=tn: xe[
                                :, k * C + te : k * C + te + tn
                            ]
                        l1_chunk(e, m, w1_sb, rhs_fn, tn, te)
                        te += tn

            gidx = 0
            for e in range(NE - 1):
                for m2 in range(KD):
                    w2_sb = w2_pool.tile([P, MQ8 * P], bf16, tag="w2")
                    nc.sync.dma_start(w2_sb[:], w2q_d[e * KD + m2])
                    layer2(e, m2, w2_sb, gidx)
                    gidx += 1
            w2_last = []
            for m2 in range(KD):
                w2_sb = w2_pool.tile([P, MQ8 * P], bf16, tag="w2")
                nc.sync.dma_start(w2_sb[:], w2q_d[(NE - 1) * KD + m2])
                w2_last.append(w2_sb)
            for m2 in range(KD):
                layer2(
                    NE - 1,
                    m2,
                    w2_last[m2],
                    gidx,
                    last=(m2 == KD - 1),
                    sync_store=True,
                )
                gidx += 1

    nc.compile()
    return nc


def _arrange_w1_slice(w1_e, lo, nblk):
    """w1 column slice [D, nblk*P] -> [nblk, P, D]."""
    q = w1_e[:, lo : lo + nblk * P].astype(BF16)
    return np.ascontiguousarray(
        q.reshape(KD, P, nblk, P).transpose(2, 1, 0, 3).reshape(nblk, P, D)
    )


def _arrange_w2_slice(w2_e, lo, nblk):
    """w2 row slice [nblk*P, D] -> [KD, P, nblk*P]."""
    q = w2_e[lo : lo + nblk * P, :].astype(BF16)
    return np.ascontiguousarray(
        q.reshape(nblk, P, KD, P).transpose(2, 1, 0, 3).reshape(KD, P, nblk * P)
    )


def _run_pass_tp8(x, w1, b1, w2, b2, idx, out):
    from concourse.bass_utils import run_bass_kernel_spmd

    global LAST_RESULT

    counts = np.array([len(i) for i in idx])
    order = np.argsort(counts, kind="stable")  # ascending: small expert first
    exps = [int(order[e]) for e in range(E)]
    caps = tuple(max(16, -(-counts[e] // 8) * 8) for e in exps)

    key = ("tp8",) + caps
    if key not in _PROGRAM_CACHE:
        _PROGRAM_CACHE[key] = _build_program_tp8(caps)
    nc = _PROGRAM_CACHE[key]

    # token arrangement is shared by all 8 cores
    common = {}
    toks0 = x[idx[exps[0]]]
    t0 = 0
    for ci, tn in enumerate(_chunk_sizes_slot0(caps[0])):
        common[f"xt0c{ci}"] = _arrange_tokens(toks0[t0 : t0 + tn], tn)
        t0 += tn
    for s in range(1, E):
        common[f"xt{s}"] = _arrange_tokens(x[idx[exps[s]]], caps[s])

    in_maps = []
    for c in range(E):
        lo = c * (MQ8 * P)
        m = dict(common)
        m["w1q"] = np.concatenate(
            [_arrange_w1_slice(w1[e], lo, MQ8) for e in exps]
        )
        m["b1q"] = np.ascontiguousarray(
            np.concatenate(
                [b1[e][lo : lo + MQ8 * P].reshape(MQ8, P).T for e in exps], axis=1
            )
        )
        m["w2q"] = np.concatenate(
            [_arrange_w2_slice(w2[e], lo, MQ8) for e in exps]
        )
        in_maps.append(m)

    res = run_bass_kernel_spmd(
        nc,
        in_maps,
        core_ids=list(range(E)),
        trace=TRACE,
        **({"trace_cores": TRACE_CORES} if TRACE_CORES else {}),
    )
    LAST_RESULT = res

    for s, e in enumerate(exps):
        n = len(idx[e])
        if n == 0:
            continue
        yt = sum(res.results[h][f"yt{s}"].astype(np.float32) for h in range(E))
        ye = yt.transpose(2, 0, 1).reshape(-1, D)
        out[idx[e]] = ye[:n] + b2[e]


def _arrange_w1_quarter(w1_e, h):
    """w1 quarter: [D, 1024] -> [MQ, P, D] with [m,p,k*128+j] = w1[k*128+p, off+m*128+j]."""
    q = w1_e[:, h * (MQ * P) : (h + 1) * (MQ * P)].astype(BF16)
    return np.ascontiguousarray(
        q.reshape(KD, P, MQ, P).transpose(2, 1, 0, 3).reshape(MQ, P, D)
    )


def _arrange_w2_quarter(w2_e, h):
    """w2 quarter: [1024, D] -> [KD, P, MQ*P] with [m2,p,k2*128+j] = w2[off+k2*128+p, m2*128+j]."""
    q = w2_e[h * (MQ * P) : (h + 1) * (MQ * P), :].astype(BF16)
    return np.ascontiguousarray(
        q.reshape(MQ, P, KD, P).transpose(2, 1, 0, 3).reshape(KD, P, MQ * P)
    )


def _arrange_tokens(x_e, C):
    """[n, D] tokens -> xt[p, k*C + c] = x_e[c, k*128 + p], zero-padded, bf16."""
    xe = np.zeros((C, D), BF16)
    xe[: len(x_e)] = x_e
    return np.ascontiguousarray(
        xe.T.reshape(KD, P, C).transpose(1, 0, 2).reshape(P, KD * C)
    )


def _run_pass_tp4(x, w1, b1, w2, b2, idx, out):
    from concourse.bass_utils import run_bass_kernel_spmd

    global LAST_RESULT

    counts = np.array([len(i) for i in idx])
    order = np.argsort(-counts, kind="stable")
    # snake pairing: rank-(2r, 2r+1) experts share a slot (one per quad) so
    # slotwise maxima are minimal; slots are processed in ASCENDING
    # capacity order (slot 0 smallest) so the DMA-gating prefix is minimal.
    pr = [3, 2, 1, 0]
    groups = [[int(order[2 * pr[j] + g]) for j in range(4)] for g in range(2)]
    caps = tuple(
        _pad_cap(max(counts[order[2 * pr[j]]], counts[order[2 * pr[j] + 1]]))
        for j in range(4)
    )

    key = ("tp4v3",) + caps
    if key not in _PROGRAM_CACHE:
        _PROGRAM_CACHE[key] = _build_program_tp4(caps)
    nc = _PROGRAM_CACHE[key]

    in_maps = []
    for c in range(E):
        g, h = divmod(c, 4)
        exps = groups[g]
        m = {
            "w1q": np.concatenate([_arrange_w1_quarter(w1[e], h) for e in exps]),
            "b1q": np.ascontiguousarray(
                np.concatenate(
                    [
                        b1[e][h * (MQ * P) : (h + 1) * (MQ * P)].reshape(MQ, P).T
                        for e in exps
                    ],
                    axis=1,
                )
            ),
            "w2q": np.concatenate([_arrange_w2_quarter(w2[e], h) for e in exps]),
        }
        for s, e in enumerate(exps):
            if s == 0:
                toks = x[idx[e]]
                t0 = 0
                for ci, tn in enumerate(_chunk_sizes_slot0(caps[0])):
                    m[f"xt0c{ci}"] = _arrange_tokens(toks[t0 : t0 + tn], tn)
                    t0 += tn
            else:
                m[f"xt{s}"] = _arrange_tokens(x[idx[e]], caps[s])
        in_maps.append(m)

    res = run_bass_kernel_spmd(
        nc,
        in_maps,
        core_ids=list(range(E)),
        trace=TRACE,
        **({"trace_cores": TRACE_CORES} if TRACE_CORES else {}),
    )
    LAST_RESULT = res

    for g in range(2):
        for s, e in enumerate(groups[g]):
            n = len(idx[e])
            if n == 0:
                continue
            yt = sum(
                res.results[4 * g + h][f"yt{s}"].astype(np.float32) for h in range(4)
            )
            ye = yt.transpose(2, 0, 1).reshape(-1, D)
            out[idx[e]] = ye[:n] + b2[e]


def kernel(x, gate_w, gate_b, w1, b1, w2, b2):
    global LAST_RESULT

    x = np.ascontiguousarray(np.asarray(x, dtype=np.float32))
    gate_w = np.asarray(gate_w, dtype=np.float32)
    gate_b = np.asarray(gate_b, dtype=np.float32)
    w1 = np.asarray(w1, dtype=np.float32)
    b1 = np.asarray(b1, dtype=np.float32)
    w2 = np.asarray(w2, dtype=np.float32)
    b2 = np.asarray(b2, dtype=np.float32)
    n_tok = x.shape[0]

    # host gate + top-1 routing (fp64: exact argmax, see module docstring)
    logits = x.astype(np.float64) @ gate_w.astype(np.float64) + gate_b.astype(
        np.float64
    )
    assign = np.argmax(logits, axis=-1)
    idx_full = [np.nonzero(assign == e)[0] for e in range(E)]

    # Defensive slabbing: if routing were pathologically imbalanced, process
    # tokens in passes so per-expert capacity stays within SBUF limits. With
    # the benchmark's near-uniform gate this is a single pass.
    slab = 600 if USE_TP8 else 960
    n_pass = max(1, -(-max(len(i) for i in idx_full) // slab))
    out = np.zeros((n_tok, D), np.float32)
    run = _run_pass_tp8 if USE_TP8 else _run_pass_tp4
    for ps in range(n_pass):
        idx = [i[ps * slab : (ps + 1) * slab] for i in idx_full]
        run(x, w1, b1, w2, b2, idx, out)
    return out


USE_TP8 = False
